# revision 1
# baseline (speedup 1.0000x reference)
"""3-layer GAT (2 heads x 128) on 8 TRN2 NeuronCores — Bass/Tile kernel.

Sharding: nodes partitioned across cores by destination (graph parallel);
weights replicated; per-layer AllGather of transposed features.

Device algorithm per layer:
  phase A' (local): esed[j] = hT_local_chunk.T @ W_ext[:,256:260]  (es/ed)
  phase A (replicated): table[p] = h[p] @ W_ext  -> [xh(256), es, ed] rows
  phase B (sharded, per 128-dst block):
    dma_gather rows by src (2 src-half buckets, int16 idx), dma_gather
    ed by dst; ex = exp(max(t, .2t)), msg = ex*xh; PE matmul with one-hot
    S tiles accumulates [128 dst, 258] (agg heads + denominators);
    out = (agg0/den0 + agg1/den1)/2 + b  (+ELU); PE-transpose -> hT shard.
  AllGather hT shards between layers. Softmax max-subtraction is skipped
  (attention logits are O(1) here; exp is safe in fp32).
"""
import dataclasses
import numpy as np

import concourse.bass as bass
import concourse.bacc as bacc
import concourse.mybir as mybir
import concourse.tile as tile

f32 = mybir.dt.float32
f32r = mybir.dt.bfloat16  # bf16 matmul operands (fp32r broken on HW)
i16 = mybir.dt.int16
ALU = mybir.AluOpType
ACTF = mybir.ActivationFunctionType


@dataclasses.dataclass(frozen=True)
class Cfg:
    n: int = 50000
    ncores: int = 8
    t_bkt: int = 10
    nlayers: int = 3
    hid: int = 128          # per-head dim == in feat dim == 128 (fixed)

    @property
    def nb(self):  return self.n // self.ncores
    @property
    def cpb(self):  return (self.nb + 127) // 128
    @property
    def npc(self):  return self.cpb * 128
    @property
    def npad(self): return self.ncores * self.npc
    @property
    def half(self): return self.npad // 2
    @property
    def tpb(self):  return 2 * self.t_bkt
    @property
    def kb(self):   return self.t_bkt * 128


ROW = 384
EROW = 64


# ---------------------------------------------------------------- host side

def pack_nodes(cfg, deg):
    """perm [N] -> padded slot id. Cores by contiguous range; within a core,
    degree-sorted snake deal into cpb blocks (balances block edge counts)."""
    perm = np.full(cfg.n, -1, dtype=np.int64)
    for c in range(cfg.ncores):
        nodes = np.arange(c * cfg.nb, (c + 1) * cfg.nb)
        order = nodes[np.argsort(-deg[nodes], kind="stable")]
        blk = np.empty(cfg.nb, dtype=np.int64)
        slot = np.empty(cfg.nb, dtype=np.int64)
        fr = cfg.nb // cfg.cpb
        rem = cfg.nb - fr * cfg.cpb
        for r in range(fr):
            cols = np.arange(cfg.cpb)
            if r % 2:
                cols = cols[::-1]
            blk[r * cfg.cpb:(r + 1) * cfg.cpb] = cols
            slot[r * cfg.cpb:(r + 1) * cfg.cpb] = r
        if rem:
            cols = np.arange(rem) if fr % 2 == 0 else (cfg.cpb - 1 - np.arange(rem))
            blk[fr * cfg.cpb:] = cols
            slot[fr * cfg.cpb:] = fr
        perm[order] = c * cfg.npc + blk * 128 + slot
    return perm


def preprocess(cfg, edge_index):
    src0 = np.asarray(edge_index[0], dtype=np.int64)
    dst0 = np.asarray(edge_index[1], dtype=np.int64)
    loop = np.arange(cfg.n, dtype=np.int64)
    src = np.concatenate([src0, loop])
    dst = np.concatenate([dst0, loop])

    deg = np.bincount(dst, minlength=cfg.n)
    perm = pack_nodes(cfg, deg)

    psrc = perm[src]
    pdst = perm[dst]
    core = pdst // cfg.npc
    blk = (pdst % cfg.npc) // 128
    half = (psrc >= cfg.half).astype(np.int64)

    order = np.lexsort((psrc, half, blk, core))
    psrc, pdst, half = psrc[order], pdst[order], half[order]
    group = (core * cfg.cpb + blk)[order] * 2 + half

    ngroups = cfg.ncores * cfg.cpb * 2
    cnt = np.bincount(group, minlength=ngroups)
    t_need = int((cnt.max() + 127) // 128)
    assert cfg.t_bkt >= t_need, f"t_bkt={cfg.t_bkt} < needed {t_need}"
    starts = np.zeros(ngroups + 1, dtype=np.int64)
    np.cumsum(cnt, out=starts[1:])
    within = np.arange(len(group)) - starts[group]
    gpos = group * cfg.kb + within

    idx1 = np.zeros(ngroups * cfg.kb, dtype=np.int16)
    idx1[gpos] = (psrc - half * cfg.half).astype(np.int16)
    idx1 = idx1.reshape(cfg.ncores, cfg.cpb, 2 * cfg.kb)

    idx2 = np.zeros(ngroups * cfg.kb, dtype=np.int16)
    idx2[gpos] = (pdst % cfg.npc).astype(np.int16)
    idx2 = idx2.reshape(cfg.ncores, cfg.cpb, 2 * cfg.kb)

    sval = np.full(ngroups * cfg.kb, -1, dtype=np.int16)
    sval[gpos] = (pdst % 128).astype(np.int16)
    sval = sval.reshape(cfg.ncores, cfg.cpb, 2 * cfg.kb)
    return dict(perm=perm, t_need=t_need, idx1=idx1, idx2=idx2, sval=sval)


def wrap_rep(idx):
    """[..., K] int16 -> dma_gather wrapped layout [128, prod*K/16]."""
    K = idx.shape[-1]
    lead = int(np.prod(idx.shape[:-1]))
    w = idx.reshape(lead, K // 16, 16)
    w = np.transpose(w, (2, 0, 1)).reshape(16, lead * (K // 16))
    return np.tile(w, (8, 1)).copy()


def host_arrays(cfg, x, edge_index, params):
    import ml_dtypes
    bfl = ml_dtypes.bfloat16
    pp = preprocess(cfg, edge_index)
    perm = pp["perm"]

    xpad = np.zeros((cfg.npad, 128), dtype=np.float32)
    xpad[perm] = np.asarray(x, np.float32)
    xT_stack = np.ascontiguousarray(
        xpad.reshape(cfg.ncores, cfg.npc, 128).transpose(0, 2, 1)
        .reshape(cfg.ncores * 128, cfg.npc))

    w_ext = np.zeros((cfg.nlayers, 128, ROW), dtype=np.float32)
    bias = np.zeros((cfg.nlayers, 128, 128), dtype=np.float32)
    for li, (W, a_s, a_d, b) in enumerate(params):
        W = np.asarray(W, np.float32)
        w_ext[li, :, :256] = W
        w_ext[li, :, 256] = W[:, :128] @ np.asarray(a_s, np.float32)[0]
        w_ext[li, :, 257] = W[:, 128:] @ np.asarray(a_s, np.float32)[1]
        w_ext[li, :, 258] = W[:, :128] @ np.asarray(a_d, np.float32)[0]
        w_ext[li, :, 259] = W[:, 128:] @ np.asarray(a_d, np.float32)[1]
        bias[li] = np.tile(np.asarray(b, np.float32)[None, :], (128, 1))

    per_core = []
    for c in range(cfg.ncores):
        sv = pp["sval"][c].astype(np.int64)                  # [cpb, 2*kb]
        S = np.zeros((cfg.cpb, 128, cfg.tpb * 128), dtype=np.float32)
        bidx, eidx = np.nonzero(sv >= 0)
        t = eidx // 128
        e = eidx % 128
        S[bidx, e, t * 128 + sv[bidx, eidx]] = 1.0
        per_core.append(dict(
            xT_stack=xT_stack.astype(bfl),
            xT_local=np.ascontiguousarray(xT_stack[c * 128:(c + 1) * 128]).astype(bfl),
            w_ext=w_ext.astype(bfl), bias=bias,
            ident=np.eye(128, dtype=np.float32),
            idx1r=wrap_rep(pp["idx1"][c]),
            idx2r=wrap_rep(pp["idx2"][c]),
            s_tiles=S.astype(bfl),
        ))
    return pp, per_core


# -------------------------------------------------------------- device side

def build_nc(cfg):
    nc = bacc.Bacc("TRN2", num_devices=cfg.ncores)
    NPC, CPB, TPB, TB, KB, HALF = (cfg.npc, cfg.cpb, cfg.tpb, cfg.t_bkt,
                                   cfg.kb, cfg.half)
    NL = cfg.nlayers
    NSH = cfg.ncores          # shards
    HSH = NSH // 2            # shards per table half

    xT_stack = nc.dram_tensor("xT_stack", [NSH * 128, NPC], f32r, kind="ExternalInput")
    xT_local = nc.dram_tensor("xT_local", [128, NPC], f32r, kind="ExternalInput")
    w_ext_in = nc.dram_tensor("w_ext", [NL, 128, ROW], f32r, kind="ExternalInput")
    bias_in = nc.dram_tensor("bias", [NL, 128, 128], f32, kind="ExternalInput")
    ident_in = nc.dram_tensor("ident", [128, 128], f32, kind="ExternalInput")
    idx1_in = nc.dram_tensor("idx1r", [128, CPB * 2 * KB // 16], i16, kind="ExternalInput")
    idx2_in = nc.dram_tensor("idx2r", [128, CPB * 2 * KB // 16], i16, kind="ExternalInput")
    s_in = nc.dram_tensor("s_tiles", [CPB, 128, TPB * 128], f32r, kind="ExternalInput")
    out = nc.dram_tensor("out", [NPC, 128], f32, kind="ExternalOutput")

    with tile.TileContext(nc) as tc:
        with (
            tc.tile_pool(name="const", bufs=1) as constp,
            tc.tile_pool(name="dram", bufs=2, space="DRAM") as dramp,
        ):
            idx1_sb = constp.tile([128, CPB * 2 * KB // 16], i16)
            nc.sync.dma_start(idx1_sb[:], idx1_in.ap())
            idx2_sb = constp.tile([128, CPB * 2 * KB // 16], i16)
            nc.sync.dma_start(idx2_sb[:], idx2_in.ap())
            w_sb = constp.tile([128, NL * ROW], f32r)
            bias_sb = constp.tile([128, NL * 128], f32)
            for li in range(NL):
                nc.sync.dma_start(w_sb[:, li * ROW:(li + 1) * ROW], w_ext_in.ap()[li])
                nc.sync.dma_start(bias_sb[:, li * 128:(li + 1) * 128], bias_in.ap()[li])
            ident_sb = constp.tile([128, 128], f32)
            nc.sync.dma_start(ident_sb[:], ident_in.ap())

            greg1 = nc.gpsimd.to_reg(KB)
            greg2 = nc.gpsimd.to_reg(2 * KB)

            hT_ag = None      # DRAM [NSH*128, NPC]; None for layer 0
            hT_loc_dram = None

            for li in range(NL):
                w_l = w_sb[:, li * ROW:(li + 1) * ROW]
                bias_l = bias_sb[:, li * 128:(li + 1) * 128]
                last = li == NL - 1

                table = [dramp.tile([HSH * NPC, ROW], f32r, tag=f"tab{h}",
                                    name=f"table_l{li}_h{h}")
                         for h in range(2)]
                esed = dramp.tile([NPC, EROW], f32, tag="esed")

                # ---- phase A': local es/ed table
                with (
                    tc.tile_pool(name="slabL", bufs=1) as slabLp,
                    tc.tile_pool(name="aeps", bufs=4) as aepsp,
                    tc.tile_pool(name="psumE", bufs=4, space="PSUM") as psumEp,
                ):
                    hTl = slabLp.tile([128, NPC], f32r)
                    if li == 0:
                        nc.sync.dma_start(hTl[:], xT_local.ap())
                    else:
                        nc.sync.dma_start(hTl[:], hT_loc_dram[:])
                    for j in range(CPB):
                        psE = psumEp.tile([128, 4], f32)
                        nc.tensor.matmul(
                            psE[:],
                            hTl[:, j * 128:(j + 1) * 128],
                            w_l[:, 256:260],
                            start=True, stop=True)
                        tE = aepsp.tile([128, 4], f32)
                        nc.vector.tensor_copy(tE[:], psE[:])
                        nc.sync.dma_start(esed[j * 128:(j + 1) * 128, 0:4], tE[:])

                # ---- phase A: full table (replicated)
                with (
                    tc.tile_pool(name="slabA", bufs=2) as slabAp,
                    tc.tile_pool(name="rowA", bufs=8) as rowAp,
                    tc.tile_pool(name="psumA", bufs=6, space="PSUM") as psumAp,
                ):
                    for s in range(NSH):
                        hTs = slabAp.tile([128, NPC], f32r)
                        if li == 0:
                            nc.sync.dma_start(
                                hTs[:], xT_stack.ap()[s * 128:(s + 1) * 128])
                        else:
                            nc.sync.dma_start(
                                hTs[:], hT_ag[s * 128:(s + 1) * 128])
                        tab = table[s // HSH]
                        base = (s % HSH) * NPC
                        for j in range(CPB):
                            psA = psumAp.tile([128, ROW], f32)
                            nc.tensor.matmul(
                                psA[:],
                                hTs[:, j * 128:(j + 1) * 128],
                                w_l,
                                start=True, stop=True)
                            tA = rowAp.tile([128, ROW], f32r)
                            nc.vector.tensor_copy(tA[:], psA[:])
                            nc.sync.dma_start(
                                tab[base + j * 128: base + (j + 1) * 128, :], tA[:])

                # ---- phase B
                with (
                    tc.tile_pool(name="g1", bufs=8) as g1p,
                    tc.tile_pool(name="g2", bufs=4) as g2p,
                    tc.tile_pool(name="sp", bufs=6) as sp,
                    tc.tile_pool(name="att", bufs=8) as attp,
                    tc.tile_pool(name="msgp", bufs=4) as msgp,
                    tc.tile_pool(name="ep", bufs=8) as epp,
                    tc.tile_pool(name="houtp", bufs=1) as houtp,
                    tc.tile_pool(name="psumB", bufs=3, space="PSUM") as psumBp,
                    tc.tile_pool(name="psumT", bufs=2, space="PSUM") as psumTp,
                ):
                    houtT = None
                    if not last:
                        houtT = houtp.tile([128, NPC], f32r)
                    for b in range(CPB):
                        psum = psumBp.tile([128, 258], f32)
                        g2 = g2p.tile([128, TPB, EROW], f32)
                        nc.gpsimd.dma_gather(
                            out_ap=g2[:], in_ap=esed,
                            idxs_ap=idx2_sb[:, b * 2 * KB // 16:
                                            (b + 1) * 2 * KB // 16],
                            num_idxs=2 * KB, num_idxs_reg=greg2,
                            elem_size=EROW, single_packet=False)
                        for h in range(2):
                            s_sb = sp.tile([128, TB * 128], f32r, name="s_sb")
                            nc.sync.dma_start(
                                s_sb[:],
                                s_in.ap()[b, :, h * TB * 128:(h + 1) * TB * 128])
                            g1 = g1p.tile([128, TB, ROW], f32r, name="g1")
                            nc.gpsimd.dma_gather(
                                out_ap=g1[:],
                                in_ap=table[h],
                                idxs_ap=idx1_sb[:, (b * 2 + h) * KB // 16:
                                                (b * 2 + h + 1) * KB // 16],
                                num_idxs=KB, num_idxs_reg=greg1,
                                elem_size=ROW, single_packet=False)
                            # attention scalars (batched across TB tiles)
                            a32 = attp.tile([128, TB, 2], f32, tag="a32")
                            nc.vector.tensor_copy(a32[:], g1[:, :, 256:258])
                            tat = attp.tile([128, TB, 2], f32, tag="tat")
                            nc.vector.tensor_tensor(
                                out=tat[:], in0=a32[:],
                                in1=g2[:, h * TB:(h + 1) * TB, 2:4], op=ALU.add)
                            lk = attp.tile([128, TB, 2], f32, tag="lk")
                            nc.vector.tensor_scalar(
                                out=lk[:], in0=tat[:], scalar1=0.2,
                                scalar2=None, op0=ALU.mult)
                            nc.vector.tensor_tensor(
                                out=lk[:], in0=lk[:], in1=tat[:], op=ALU.max)
                            exe = attp.tile([128, TB, 2], f32, tag="exe")
                            nc.scalar.activation(exe[:], lk[:], ACTF.Exp)
                            exb = attp.tile([128, TB, 2], f32r, tag="exb")
                            nc.vector.tensor_copy(exb[:], exe[:])
                            msg = msgp.tile([128, TB, 258], f32r, name="msg")
                            for hh in range(2):
                                nc.vector.tensor_tensor(
                                    out=msg[:, :, hh * 128:(hh + 1) * 128],
                                    in0=g1[:, :, hh * 128:(hh + 1) * 128],
                                    in1=exb[:, :, hh:hh + 1].broadcast_to(
                                        (128, TB, 128)),
                                    op=ALU.mult)
                            nc.vector.tensor_copy(msg[:, :, 256:258], exb[:])
                            for t in range(TB):
                                nc.tensor.matmul(
                                    psum[:],
                                    s_sb[:, t * 128:(t + 1) * 128],
                                    msg[:, t, :],
                                    start=(h == 0 and t == 0),
                                    stop=(h == 1 and t == TB - 1))
                        # epilogue
                        rec = attp.tile([128, 2], f32, tag="rec")
                        nc.vector.tensor_scalar(
                            out=rec[:], in0=psum[:, 256:258], scalar1=1e-30,
                            scalar2=None, op0=ALU.add)
                        nc.vector.reciprocal(rec[:], rec[:])
                        h_blk = epp.tile([128, 128], f32, tag="hblk")
                        nc.vector.tensor_scalar(
                            out=h_blk[:], in0=psum[:, 0:128],
                            scalar1=rec[:, 0:1], scalar2=0.5,
                            op0=ALU.mult, op1=ALU.mult)
                        m1 = epp.tile([128, 128], f32, tag="m1")
                        nc.vector.tensor_scalar(
                            out=m1[:], in0=psum[:, 128:256],
                            scalar1=rec[:, 1:2], scalar2=0.5,
                            op0=ALU.mult, op1=ALU.mult)
                        nc.vector.tensor_tensor(
                            out=h_blk[:], in0=h_blk[:], in1=m1[:], op=ALU.add)
                        nc.vector.tensor_tensor(
                            out=h_blk[:], in0=h_blk[:], in1=bias_l, op=ALU.add)
                        if not last:
                            # ELU = relu(x) + exp(min(x,0)) - 1
                            mn = epp.tile([128, 128], f32, tag="mn")
                            nc.vector.tensor_scalar(
                                out=mn[:], in0=h_blk[:], scalar1=0.0,
                                scalar2=None, op0=ALU.min)
                            emn = epp.tile([128, 128], f32, tag="emn")
                            nc.scalar.activation(emn[:], mn[:], ACTF.Exp)
                            nc.vector.tensor_scalar(
                                out=h_blk[:], in0=h_blk[:], scalar1=0.0,
                                scalar2=None, op0=ALU.max)
                            nc.vector.tensor_tensor(
                                out=h_blk[:], in0=h_blk[:], in1=emn[:],
                                op=ALU.add)
                            nc.vector.tensor_scalar(
                                out=h_blk[:], in0=h_blk[:], scalar1=-1.0,
                                scalar2=None, op0=ALU.add)
                            psT = psumTp.tile([128, 128], f32)
                            nc.tensor.transpose(psT[:], h_blk[:], ident_sb[:])
                            nc.vector.tensor_copy(
                                houtT[:, b * 128:(b + 1) * 128], psT[:])
                        else:
                            nc.sync.dma_start(
                                out[b * 128:(b + 1) * 128, :], h_blk[:])
                    if not last:
                        hT_loc_dram = dramp.tile([128, NPC], f32r, tag="hloc")
                        nc.sync.dma_start(hT_loc_dram[:], houtT[:])
                        hT_ag = dramp.tile([NSH * 128, NPC], f32r, tag="hag", addr_space="Shared")
                        nc.gpsimd.collective_compute(
                            "AllGather", ALU.bypass,
                            replica_groups=[list(range(cfg.ncores))],
                            ins=[hT_loc_dram.opt()], outs=[hT_ag.opt()])
    nc.compile()
    return nc


# ------------------------------------------------------------------ driver

def run(cfg, x, edge_index, params, trace=False):
    from concourse.bass_utils import run_bass_kernel_spmd
    pp, per_core = host_arrays(cfg, x, edge_index, params)
    nc = build_nc(cfg)
    in_maps = [
        dict(xT_stack=pc["xT_stack"], xT_local=pc["xT_local"],
             w_ext=pc["w_ext"], bias=pc["bias"], ident=pc["ident"],
             idx1r=pc["idx1r"], idx2r=pc["idx2r"], s_tiles=pc["s_tiles"])
        for pc in per_core
    ]
    res = run_bass_kernel_spmd(
        nc, in_maps, core_ids=list(range(cfg.ncores)), trace=trace)
    full = np.concatenate([res.results[c]["out"] for c in range(cfg.ncores)])
    return full[pp["perm"]], res


# ------------------------------------------------------------- entry point

_CFG = Cfg()


def kernel(x, edge_index, W0, a_src0, a_dst0, b0, W1, a_src1, a_dst1, b1,
           W2, a_src2, a_dst2, b2):
    """Full-input GAT kernel: shards across 8 NeuronCores internally."""
    params = [(W0, a_src0, a_dst0, b0), (W1, a_src1, a_dst1, b1),
              (W2, a_src2, a_dst2, b2)]
    cfg = _CFG
    try:
        out, _ = run(cfg, x, edge_index, params, trace=False)
    except AssertionError:
        pp = preprocess(dataclasses.replace(cfg, t_bkt=64), edge_index)
        cfg = dataclasses.replace(cfg, t_bkt=pp["t_need"])
        out, _ = run(cfg, x, edge_index, params, trace=False)
    return np.asarray(out, dtype=np.float32)



# revision 6
# speedup vs baseline: 1.4718x; 1.4718x over previous
"""3-layer GAT (2 heads x 128) on 8 TRN2 NeuronCores — Bass/Tile kernel.

Sharding: nodes partitioned across cores by destination (graph parallel);
weights replicated; per-layer AllGather of transposed features.

Device algorithm per layer:
  phase A' (local): esed_sb[:, j, :] = hT_local_chunk.T @ W_ext[:,256:260]
    kept in SBUF (per-node es/ed of the local shard).
  phase A (replicated): table[p] = h[p] @ W_ext -> [xh(256), es, ed] rows
    (ROW=320, bf16) written to local DRAM.
  phase B (sharded, per 128-dst block):
    ed per edge slot via transposed-one-hot PE matmuls (ST tiles);
    dma_gather table rows by src (2 src-half buckets, int16 idx, exact
    valid-count register, -1 tail padding); ex = exp(lrelu(es+ed));
    msg = ex*xh; PE matmul with one-hot S tiles accumulates
    [128 dst, 258] (agg heads + denominators);
    out = (agg0/den0 + agg1/den1)/2 + b  (+ELU); PE-transpose -> hT shard.
  AllGather hT shards between layers. Softmax max-subtraction is skipped
  (attention logits are O(1) here; exp is safe in fp32).
"""
import dataclasses
import numpy as np

import concourse.bass as bass
import concourse.bacc as bacc
import concourse.mybir as mybir
import concourse.tile as tile

f32 = mybir.dt.float32
f32r = mybir.dt.bfloat16  # bf16 matmul operands (fp32r broken on HW)
i16 = mybir.dt.int16
ALU = mybir.AluOpType
ACTF = mybir.ActivationFunctionType


@dataclasses.dataclass(frozen=True)
class Cfg:
    n: int = 50000
    ncores: int = 8
    t_bkt: int = 10
    nlayers: int = 3
    hid: int = 128          # per-head dim == in feat dim == 128 (fixed)

    @property
    def nb(self):  return self.n // self.ncores
    @property
    def cpb(self):  return (self.nb + 127) // 128
    @property
    def npc(self):  return self.cpb * 128
    @property
    def npad(self): return self.ncores * self.npc
    @property
    def half(self): return self.npad // 2
    @property
    def tpb(self):  return 2 * self.t_bkt
    @property
    def kb(self):   return self.t_bkt * 128


ROW = 384


# ---------------------------------------------------------------- host side

def pack_nodes(cfg, deg):
    """perm [N] -> padded slot id. Cores by contiguous range; within a core,
    degree-sorted snake deal into cpb blocks (balances block edge counts)."""
    perm = np.full(cfg.n, -1, dtype=np.int64)
    for c in range(cfg.ncores):
        nodes = np.arange(c * cfg.nb, (c + 1) * cfg.nb)
        order = nodes[np.argsort(-deg[nodes], kind="stable")]
        blk = np.empty(cfg.nb, dtype=np.int64)
        slot = np.empty(cfg.nb, dtype=np.int64)
        fr = cfg.nb // cfg.cpb
        rem = cfg.nb - fr * cfg.cpb
        for r in range(fr):
            cols = np.arange(cfg.cpb)
            if r % 2:
                cols = cols[::-1]
            blk[r * cfg.cpb:(r + 1) * cfg.cpb] = cols
            slot[r * cfg.cpb:(r + 1) * cfg.cpb] = r
        if rem:
            cols = np.arange(rem) if fr % 2 == 0 else (cfg.cpb - 1 - np.arange(rem))
            blk[fr * cfg.cpb:] = cols
            slot[fr * cfg.cpb:] = fr
        perm[order] = c * cfg.npc + blk * 128 + slot
    return perm


def preprocess(cfg, edge_index):
    src0 = np.asarray(edge_index[0], dtype=np.int64)
    dst0 = np.asarray(edge_index[1], dtype=np.int64)
    loop = np.arange(cfg.n, dtype=np.int64)
    src = np.concatenate([src0, loop])
    dst = np.concatenate([dst0, loop])

    deg = np.bincount(dst, minlength=cfg.n)
    perm = pack_nodes(cfg, deg)

    psrc = perm[src]
    pdst = perm[dst]
    core = pdst // cfg.npc
    blk = (pdst % cfg.npc) // 128
    half = (psrc >= cfg.half).astype(np.int64)

    order = np.lexsort((psrc, half, blk, core))
    psrc, pdst, half = psrc[order], pdst[order], half[order]
    group = (core * cfg.cpb + blk)[order] * 2 + half

    ngroups = cfg.ncores * cfg.cpb * 2
    cnt = np.bincount(group, minlength=ngroups)
    t_need = int((cnt.max() + 127) // 128)
    assert cfg.t_bkt >= t_need, f"t_bkt={cfg.t_bkt} < needed {t_need}"
    starts = np.zeros(ngroups + 1, dtype=np.int64)
    np.cumsum(cnt, out=starts[1:])
    within = np.arange(len(group)) - starts[group]
    gpos = group * cfg.kb + within

    # per-(block, half) valid-count C = max over cores, rounded up to 32.
    # Slots [cnt, C) are duplicate idx 0 (valid; sval=-1 kills them in S/ST);
    # slots [C, kb) are -1 and generate no DMA descriptors.
    cnt_cbh = cnt.reshape(cfg.ncores, cfg.cpb, 2)
    C = cnt_cbh.max(axis=0)
    C = np.minimum((C + 31) // 32 * 32, cfg.kb)            # [cpb, 2]

    slot_w = np.arange(ngroups * cfg.kb) % cfg.kb
    Cg = np.broadcast_to(C[None, :, :], (cfg.ncores, cfg.cpb, 2)).reshape(ngroups)

    idx1 = np.full(ngroups * cfg.kb, -1, dtype=np.int16)
    idx1[slot_w < Cg[np.arange(ngroups * cfg.kb) // cfg.kb]] = 0
    idx1[gpos] = (psrc - half * cfg.half).astype(np.int16)
    idx1 = idx1.reshape(cfg.ncores, cfg.cpb, 2 * cfg.kb)

    sval = np.full(ngroups * cfg.kb, -1, dtype=np.int16)
    sval[gpos] = (pdst % 128).astype(np.int16)
    sval = sval.reshape(cfg.ncores, cfg.cpb, 2 * cfg.kb)
    return dict(perm=perm, t_need=t_need, idx1=idx1, sval=sval, C=C)


def wrap_rep(idx):
    """[..., K] int16 -> dma_gather wrapped layout [128, prod*K/16]."""
    K = idx.shape[-1]
    lead = int(np.prod(idx.shape[:-1]))
    w = idx.reshape(lead, K // 16, 16)
    w = np.transpose(w, (2, 0, 1)).reshape(16, lead * (K // 16))
    return np.tile(w, (8, 1)).copy()


def host_arrays(cfg, x, edge_index, params):
    import ml_dtypes
    bfl = ml_dtypes.bfloat16
    pp = preprocess(cfg, edge_index)
    perm = pp["perm"]

    xpad = np.zeros((cfg.npad, 128), dtype=np.float32)
    xpad[perm] = np.asarray(x, np.float32)
    xT_stack = np.ascontiguousarray(
        xpad.reshape(cfg.ncores, cfg.npc, 128).transpose(0, 2, 1)
        .reshape(cfg.ncores * 128, cfg.npc))

    w_ext = np.zeros((cfg.nlayers, 128, ROW), dtype=np.float32)
    bias = np.zeros((cfg.nlayers, 128, 128), dtype=np.float32)
    for li, (W, a_s, a_d, b) in enumerate(params):
        W = np.asarray(W, np.float32)
        w_ext[li, :, :256] = W
        w_ext[li, :, 256] = W[:, :128] @ np.asarray(a_s, np.float32)[0]
        w_ext[li, :, 257] = W[:, 128:] @ np.asarray(a_s, np.float32)[1]
        w_ext[li, :, 258] = W[:, :128] @ np.asarray(a_d, np.float32)[0]
        w_ext[li, :, 259] = W[:, 128:] @ np.asarray(a_d, np.float32)[1]
        bias[li] = np.tile(np.asarray(b, np.float32)[None, :], (128, 1))

    per_core = []
    for c in range(cfg.ncores):
        sv = pp["sval"][c].astype(np.int64)                  # [cpb, 2*kb]
        bidx, eidx = np.nonzero(sv >= 0)
        t = eidx // 128
        e = eidx % 128
        S = np.zeros((cfg.cpb, 128, cfg.tpb * 128), dtype=np.float32)
        S[bidx, e, t * 128 + sv[bidx, eidx]] = 1.0
        ST = np.zeros((cfg.cpb, 128, cfg.tpb * 128), dtype=np.float32)
        ST[bidx, sv[bidx, eidx], eidx] = 1.0
        per_core.append(dict(
            xT_stack=xT_stack.astype(bfl),
            xT_local=np.ascontiguousarray(xT_stack[c * 128:(c + 1) * 128]).astype(bfl),
            w_ext=w_ext.astype(bfl), bias=bias,
            ident=np.eye(128, dtype=np.float32),
            idx1r=wrap_rep(pp["idx1"][c]),
            s_tiles=S.astype(bfl),
            st_tiles=ST.astype(bfl),
        ))
    return pp, per_core


# -------------------------------------------------------------- device side

def build_nc(cfg, C):
    nc = bacc.Bacc("TRN2", num_devices=cfg.ncores)
    NPC, CPB, TPB, TB, KB = cfg.npc, cfg.cpb, cfg.tpb, cfg.t_bkt, cfg.kb
    NL = cfg.nlayers
    NSH = cfg.ncores          # shards
    HSH = NSH // 2            # shards per table half

    xT_stack = nc.dram_tensor("xT_stack", [NSH * 128, NPC], f32r, kind="ExternalInput")
    xT_local = nc.dram_tensor("xT_local", [128, NPC], f32r, kind="ExternalInput")
    w_ext_in = nc.dram_tensor("w_ext", [NL, 128, ROW], f32r, kind="ExternalInput")
    bias_in = nc.dram_tensor("bias", [NL, 128, 128], f32, kind="ExternalInput")
    ident_in = nc.dram_tensor("ident", [128, 128], f32, kind="ExternalInput")
    idx1_in = nc.dram_tensor("idx1r", [128, CPB * 2 * KB // 16], i16, kind="ExternalInput")
    s_in = nc.dram_tensor("s_tiles", [CPB, 128, TPB * 128], f32r, kind="ExternalInput")
    st_in = nc.dram_tensor("st_tiles", [CPB, 128, TPB * 128], f32r, kind="ExternalInput")
    out = nc.dram_tensor("out", [NPC, 128], f32, kind="ExternalOutput")

    with tile.TileContext(nc) as tc:
        with (
            tc.tile_pool(name="const", bufs=1) as constp,
            tc.tile_pool(name="dram", bufs=2, space="DRAM") as dramp,
            tc.tile_pool(name="g1", bufs=4) as g1p,
            tc.tile_pool(name="sp", bufs=4) as sp,
            tc.tile_pool(name="stp", bufs=4) as stp,
            tc.tile_pool(name="att", bufs=8) as attp,
            tc.tile_pool(name="msgp", bufs=4) as msgp,
            tc.tile_pool(name="edslp", bufs=4) as edslp,
            tc.tile_pool(name="ep", bufs=8) as epp,
            tc.tile_pool(name="houtp", bufs=1) as houtp,
        ):
            idx1_sb = constp.tile([128, CPB * 2 * KB // 16], i16)
            nc.sync.dma_start(idx1_sb[:], idx1_in.ap())
            w_sb = constp.tile([128, NL * ROW], f32r)
            bias_sb = constp.tile([128, NL * 128], f32)
            for li in range(NL):
                nc.sync.dma_start(w_sb[:, li * ROW:(li + 1) * ROW], w_ext_in.ap()[li])
                nc.sync.dma_start(bias_sb[:, li * 128:(li + 1) * 128], bias_in.ap()[li])
            ident_sb = constp.tile([128, 128], f32)
            nc.sync.dma_start(ident_sb[:], ident_in.ap())
            esed_sb = constp.tile([128, CPB * 4], f32r)

            # exact valid-count registers (few distinct values; rounded to 32)
            regs = {}
            for b in range(CPB):
                for h in range(2):
                    v = int(C[b][h])
                    if v not in regs:
                        regs[v] = nc.gpsimd.to_reg(v)

            # zero the gather pool slots once: tail slots beyond the valid
            # count keep stale SBUF contents; uninitialized bits could be NaN
            # bf16 patterns that poison 0*NaN in PSUM accumulation.
            for _ in range(4):
                gz = g1p.tile([128, TB, ROW], f32r, name="g1")
                nc.vector.memset(gz[:], 0)

            hT_ag = None      # DRAM [NSH*128, NPC]; None for layer 0
            hT_loc_dram = None

            for li in range(NL):
                w_l = w_sb[:, li * ROW:(li + 1) * ROW]
                bias_l = bias_sb[:, li * 128:(li + 1) * 128]
                last = li == NL - 1

                table = [dramp.tile([HSH * NPC, ROW], f32r, tag=f"tab{h}",
                                    name=f"table_l{li}_h{h}")
                         for h in range(2)]

                # ---- phase A': local es/ed kept in SBUF
                with (
                    tc.tile_pool(name="slabL", bufs=1) as slabLp,
                    tc.tile_pool(name="psumE", bufs=4, space="PSUM") as psumEp,
                ):
                    hTl = slabLp.tile([128, NPC], f32r)
                    if li == 0:
                        nc.sync.dma_start(hTl[:], xT_local.ap())
                    else:
                        nc.sync.dma_start(hTl[:], hT_loc_dram[:])
                    for j in range(CPB):
                        psE = psumEp.tile([128, 4], f32)
                        nc.tensor.matmul(
                            psE[:],
                            hTl[:, j * 128:(j + 1) * 128],
                            w_l[:, 256:260],
                            start=True, stop=True)
                        nc.scalar.activation(
                            esed_sb[:, j * 4:(j + 1) * 4], psE[:], ACTF.Copy)

                # ---- phase A: full table (replicated)
                with (
                    tc.tile_pool(name="slabA", bufs=2) as slabAp,
                    tc.tile_pool(name="rowA", bufs=8) as rowAp,
                    tc.tile_pool(name="psumA", bufs=6, space="PSUM") as psumAp,
                ):
                    for s in range(NSH):
                        hTs = slabAp.tile([128, NPC], f32r)
                        if li == 0:
                            nc.sync.dma_start(
                                hTs[:], xT_stack.ap()[s * 128:(s + 1) * 128])
                        else:
                            nc.sync.dma_start(
                                hTs[:], hT_ag[s * 128:(s + 1) * 128])
                        tab = table[s // HSH]
                        base = (s % HSH) * NPC
                        for j in range(CPB):
                            psA = psumAp.tile([128, ROW], f32)
                            nc.tensor.matmul(
                                psA[:],
                                hTs[:, j * 128:(j + 1) * 128],
                                w_l,
                                start=True, stop=True)
                            tA = rowAp.tile([128, ROW], f32r)
                            nc.scalar.activation(tA[:], psA[:], ACTF.Copy)
                            nc.sync.dma_start(
                                tab[base + j * 128: base + (j + 1) * 128, :], tA[:])

                # ---- phase B
                with (
                    tc.tile_pool(name="psumB", bufs=3, space="PSUM") as psumBp,
                    tc.tile_pool(name="psumT", bufs=2, space="PSUM") as psumTp,
                    tc.tile_pool(name="psumD", bufs=2, space="PSUM") as psumDp,
                ):
                    houtT = None
                    if not last:
                        houtT = houtp.tile([128, NPC], f32r)
                    for b in range(CPB):
                        st_sb = stp.tile([128, TPB * 128], f32r, name="st_sb")
                        nc.sync.dma_start(st_sb[:], st_in.ap()[b])
                        # ed per edge slot: transposed-one-hot matmuls
                        psD = psumDp.tile([128, TPB, 2], f32)
                        for t in range(TPB):
                            nc.tensor.matmul(
                                psD[:, t, :],
                                st_sb[:, t * 128:(t + 1) * 128],
                                esed_sb[:, b * 4 + 2:b * 4 + 4],
                                start=True, stop=True)
                        edsl = edslp.tile([128, TPB, 2], f32, name="edsl")
                        nc.scalar.activation(edsl[:], psD[:], ACTF.Copy)

                        psum = psumBp.tile([128, 258], f32)
                        for h in range(2):
                            s_sb = sp.tile([128, TB * 128], f32r, name="s_sb")
                            nc.sync.dma_start(
                                s_sb[:],
                                s_in.ap()[b, :, h * TB * 128:(h + 1) * TB * 128])
                            g1 = g1p.tile([128, TB, ROW], f32r, name="g1")
                            nc.gpsimd.dma_gather(
                                out_ap=g1[:],
                                in_ap=table[h],
                                idxs_ap=idx1_sb[:, (b * 2 + h) * KB // 16:
                                                (b * 2 + h + 1) * KB // 16],
                                num_idxs=KB, num_idxs_reg=regs[int(C[b][h])],
                                elem_size=ROW, single_packet=False)
                            # attention scalars (batched across TB tiles)
                            a32 = attp.tile([128, TB, 2], f32, tag="a32")
                            nc.vector.tensor_copy(a32[:], g1[:, :, 256:258])
                            tat = attp.tile([128, TB, 2], f32, tag="tat")
                            nc.vector.tensor_tensor(
                                out=tat[:], in0=a32[:],
                                in1=edsl[:, h * TB:(h + 1) * TB, :], op=ALU.add)
                            lk = attp.tile([128, TB, 2], f32, tag="lk")
                            nc.vector.tensor_scalar(
                                out=lk[:], in0=tat[:], scalar1=0.2,
                                scalar2=None, op0=ALU.mult)
                            nc.vector.tensor_tensor(
                                out=lk[:], in0=lk[:], in1=tat[:], op=ALU.max)
                            exe = attp.tile([128, TB, 2], f32, tag="exe")
                            nc.scalar.activation(exe[:], lk[:], ACTF.Exp)
                            exb = attp.tile([128, TB, 2], f32r, tag="exb")
                            nc.vector.tensor_copy(exb[:], exe[:])
                            msg = msgp.tile([128, TB, 258], f32r, name="msg")
                            for hh in range(2):
                                nc.vector.tensor_tensor(
                                    out=msg[:, :, hh * 128:(hh + 1) * 128],
                                    in0=g1[:, :, hh * 128:(hh + 1) * 128],
                                    in1=exb[:, :, hh:hh + 1].broadcast_to(
                                        (128, TB, 128)),
                                    op=ALU.mult)
                            nc.vector.tensor_copy(msg[:, :, 256:258], exb[:])
                            for t in range(TB):
                                nc.tensor.matmul(
                                    psum[:],
                                    s_sb[:, t * 128:(t + 1) * 128],
                                    msg[:, t, :],
                                    start=(h == 0 and t == 0),
                                    stop=(h == 1 and t == TB - 1))
                        # epilogue
                        rec = attp.tile([128, 2], f32, tag="rec")
                        nc.vector.tensor_scalar(
                            out=rec[:], in0=psum[:, 256:258], scalar1=1e-30,
                            scalar2=None, op0=ALU.add)
                        nc.vector.reciprocal(rec[:], rec[:])
                        h_blk = epp.tile([128, 128], f32, tag="hblk")
                        nc.vector.tensor_scalar(
                            out=h_blk[:], in0=psum[:, 0:128],
                            scalar1=rec[:, 0:1], scalar2=0.5,
                            op0=ALU.mult, op1=ALU.mult)
                        m1 = epp.tile([128, 128], f32, tag="m1")
                        nc.vector.tensor_scalar(
                            out=m1[:], in0=psum[:, 128:256],
                            scalar1=rec[:, 1:2], scalar2=0.5,
                            op0=ALU.mult, op1=ALU.mult)
                        nc.vector.tensor_tensor(
                            out=h_blk[:], in0=h_blk[:], in1=m1[:], op=ALU.add)
                        nc.vector.tensor_tensor(
                            out=h_blk[:], in0=h_blk[:], in1=bias_l, op=ALU.add)
                        if not last:
                            # ELU = relu(x) + exp(min(x,0)) - 1
                            mn = epp.tile([128, 128], f32, tag="mn")
                            nc.vector.tensor_scalar(
                                out=mn[:], in0=h_blk[:], scalar1=0.0,
                                scalar2=None, op0=ALU.min)
                            emn = epp.tile([128, 128], f32, tag="emn")
                            nc.scalar.activation(emn[:], mn[:], ACTF.Exp)
                            nc.vector.tensor_scalar(
                                out=h_blk[:], in0=h_blk[:], scalar1=0.0,
                                scalar2=None, op0=ALU.max)
                            nc.vector.tensor_tensor(
                                out=h_blk[:], in0=h_blk[:], in1=emn[:],
                                op=ALU.add)
                            nc.vector.tensor_scalar(
                                out=h_blk[:], in0=h_blk[:], scalar1=-1.0,
                                scalar2=None, op0=ALU.add)
                            psT = psumTp.tile([128, 128], f32)
                            nc.tensor.transpose(psT[:], h_blk[:], ident_sb[:])
                            nc.scalar.activation(
                                houtT[:, b * 128:(b + 1) * 128], psT[:],
                                ACTF.Copy)
                        else:
                            nc.sync.dma_start(
                                out[b * 128:(b + 1) * 128, :], h_blk[:])
                    if not last:
                        hT_loc_dram = dramp.tile([128, NPC], f32r, tag="hloc")
                        nc.sync.dma_start(hT_loc_dram[:], houtT[:])
                        hT_ag = dramp.tile([NSH * 128, NPC], f32r, tag="hag", addr_space="Shared")
                        nc.gpsimd.collective_compute(
                            "AllGather", ALU.bypass,
                            replica_groups=[list(range(cfg.ncores))],
                            ins=[hT_loc_dram.opt()], outs=[hT_ag.opt()])
    nc.compile()
    return nc


# ------------------------------------------------------------------ driver

def run(cfg, x, edge_index, params, trace=False):
    from concourse.bass_utils import run_bass_kernel_spmd
    pp, per_core = host_arrays(cfg, x, edge_index, params)
    nc = build_nc(cfg, pp["C"])
    in_maps = [
        dict(xT_stack=pc["xT_stack"], xT_local=pc["xT_local"],
             w_ext=pc["w_ext"], bias=pc["bias"], ident=pc["ident"],
             idx1r=pc["idx1r"], s_tiles=pc["s_tiles"], st_tiles=pc["st_tiles"])
        for pc in per_core
    ]
    res = run_bass_kernel_spmd(
        nc, in_maps, core_ids=list(range(cfg.ncores)), trace=trace)
    full = np.concatenate([res.results[c]["out"] for c in range(cfg.ncores)])
    return full[pp["perm"]], res


# ------------------------------------------------------------- entry point

_CFG = Cfg()


def kernel(x, edge_index, W0, a_src0, a_dst0, b0, W1, a_src1, a_dst1, b1,
           W2, a_src2, a_dst2, b2):
    """Full-input GAT kernel: shards across 8 NeuronCores internally."""
    params = [(W0, a_src0, a_dst0, b0), (W1, a_src1, a_dst1, b1),
              (W2, a_src2, a_dst2, b2)]
    cfg = _CFG
    try:
        out, _ = run(cfg, x, edge_index, params, trace=False)
    except AssertionError:
        pp = preprocess(dataclasses.replace(cfg, t_bkt=64), edge_index)
        cfg = dataclasses.replace(cfg, t_bkt=pp["t_need"])
        out, _ = run(cfg, x, edge_index, params, trace=False)
    return np.asarray(out, dtype=np.float32)


# revision 7
# speedup vs baseline: 1.7422x; 1.1837x over previous
"""3-layer GAT (2 heads x 128) on 8 TRN2 NeuronCores — Bass/Tile kernel.

Sharding: nodes partitioned across cores by destination (graph parallel);
weights replicated; per-layer split AllGather of transposed features.

Device algorithm per layer:
  phase A' (local): esed_sb[:, j, :] = hT_local_chunk.T @ W_ext[:,256:260]
    kept in SBUF (per-node es/ed of the local shard).
  phase A (replicated): table rows [xh(256), es(2)] (258 of ROW=384, bf16)
    written to local DRAM in 4-chunk batched DMAs.
  phase B (sharded, per 128-dst block):
    ed per edge slot via transposed-one-hot PE matmuls (ST tiles);
    dma_gather table rows by src (2 src-half buckets, int16 idx);
    ex = exp(max(t,.2t)); msg = ex*xh; PE matmul with one-hot S tiles
    accumulates [128 dst, 258] (agg heads + denominators). Self-loop
    edges never gathered: xh_local from a direct PE matmul, merged in
    the epilogue. out = (agg0/den0 + agg1/den1)/2 + b (+ELU);
    PE-transpose -> hT shard kept in SBUF (houtT, double-buffered).
  AllGather of hT split in two column halves; the first overlaps the
  tail of phase B. Softmax max-subtraction is skipped (logits are O(1);
  exp is safe in fp32).
"""
import dataclasses
import numpy as np

import concourse.bass as bass
import concourse.bacc as bacc
import concourse.mybir as mybir
import concourse.tile as tile

f32 = mybir.dt.float32
f32r = mybir.dt.bfloat16  # bf16 matmul operands (fp32r broken on HW)
i16 = mybir.dt.int16
ALU = mybir.AluOpType
ACTF = mybir.ActivationFunctionType


@dataclasses.dataclass(frozen=True)
class Cfg:
    n: int = 50000
    ncores: int = 8
    t_bkt: int = 9
    nlayers: int = 3
    hid: int = 128          # per-head dim == in feat dim == 128 (fixed)

    @property
    def nb(self):  return self.n // self.ncores
    @property
    def cpb(self):  return (self.nb + 127) // 128
    @property
    def npc(self):  return self.cpb * 128
    @property
    def npad(self): return self.ncores * self.npc
    @property
    def half(self): return self.npad // 2
    @property
    def tpb(self):  return 2 * self.t_bkt
    @property
    def kb(self):   return self.t_bkt * 128


ROW = 384
TCOL = 258                  # meaningful table cols: xh(256) + es(2)


# ---------------------------------------------------------------- host side

def pack_nodes(cfg, deg):
    """perm [N] -> padded slot id. Cores by contiguous range; within a core,
    degree-sorted snake deal into cpb blocks (balances block edge counts)."""
    perm = np.full(cfg.n, -1, dtype=np.int64)
    for c in range(cfg.ncores):
        nodes = np.arange(c * cfg.nb, (c + 1) * cfg.nb)
        order = nodes[np.argsort(-deg[nodes], kind="stable")]
        blk = np.empty(cfg.nb, dtype=np.int64)
        slot = np.empty(cfg.nb, dtype=np.int64)
        fr = cfg.nb // cfg.cpb
        rem = cfg.nb - fr * cfg.cpb
        for r in range(fr):
            cols = np.arange(cfg.cpb)
            if r % 2:
                cols = cols[::-1]
            blk[r * cfg.cpb:(r + 1) * cfg.cpb] = cols
            slot[r * cfg.cpb:(r + 1) * cfg.cpb] = r
        if rem:
            cols = np.arange(rem) if fr % 2 == 0 else (cfg.cpb - 1 - np.arange(rem))
            blk[fr * cfg.cpb:] = cols
            slot[fr * cfg.cpb:] = fr
        perm[order] = c * cfg.npc + blk * 128 + slot
    return perm


def preprocess(cfg, edge_index):
    # self-loops (appended by the reference for every node) are handled by
    # a local compute path on device, NOT via gather buckets.
    src = np.asarray(edge_index[0], dtype=np.int64)
    dst = np.asarray(edge_index[1], dtype=np.int64)

    deg = np.bincount(dst, minlength=cfg.n)
    perm = pack_nodes(cfg, deg)

    psrc = perm[src]
    pdst = perm[dst]
    core = pdst // cfg.npc
    blk = (pdst % cfg.npc) // 128
    half = (psrc >= cfg.half).astype(np.int64)

    order = np.lexsort((psrc, half, blk, core))
    psrc, pdst, half = psrc[order], pdst[order], half[order]
    group = (core * cfg.cpb + blk)[order] * 2 + half

    ngroups = cfg.ncores * cfg.cpb * 2
    cnt = np.bincount(group, minlength=ngroups)
    t_need = int((cnt.max() + 127) // 128)
    assert cfg.t_bkt >= t_need, f"t_bkt={cfg.t_bkt} < needed {t_need}"
    starts = np.zeros(ngroups + 1, dtype=np.int64)
    np.cumsum(cnt, out=starts[1:])
    within = np.arange(len(group)) - starts[group]
    gpos = group * cfg.kb + within

    idx1 = np.zeros(ngroups * cfg.kb, dtype=np.int16)
    idx1[gpos] = (psrc - half * cfg.half).astype(np.int16)
    idx1 = idx1.reshape(cfg.ncores, cfg.cpb, 2 * cfg.kb)

    sval = np.full(ngroups * cfg.kb, -1, dtype=np.int16)
    sval[gpos] = (pdst % 128).astype(np.int16)
    sval = sval.reshape(cfg.ncores, cfg.cpb, 2 * cfg.kb)
    return dict(perm=perm, t_need=t_need, idx1=idx1, sval=sval)


def wrap_rep(idx):
    """[..., K] int16 -> dma_gather wrapped layout [128, prod*K/16]."""
    K = idx.shape[-1]
    lead = int(np.prod(idx.shape[:-1]))
    w = idx.reshape(lead, K // 16, 16)
    w = np.transpose(w, (2, 0, 1)).reshape(16, lead * (K // 16))
    return np.tile(w, (8, 1)).copy()


def host_arrays(cfg, pp, x, params):
    import ml_dtypes
    bfl = ml_dtypes.bfloat16
    perm = pp["perm"]

    xpad = np.zeros((cfg.npad, 128), dtype=np.float32)
    xpad[perm] = np.asarray(x, np.float32)
    xT_stack = np.ascontiguousarray(
        xpad.reshape(cfg.ncores, cfg.npc, 128).transpose(0, 2, 1)
        .reshape(cfg.ncores * 128, cfg.npc))

    w_ext = np.zeros((cfg.nlayers, 128, ROW), dtype=np.float32)
    bias = np.zeros((cfg.nlayers, 128, 128), dtype=np.float32)
    for li, (W, a_s, a_d, b) in enumerate(params):
        W = np.asarray(W, np.float32)
        w_ext[li, :, :256] = W
        w_ext[li, :, 256] = W[:, :128] @ np.asarray(a_s, np.float32)[0]
        w_ext[li, :, 257] = W[:, 128:] @ np.asarray(a_s, np.float32)[1]
        w_ext[li, :, 258] = W[:, :128] @ np.asarray(a_d, np.float32)[0]
        w_ext[li, :, 259] = W[:, 128:] @ np.asarray(a_d, np.float32)[1]
        bias[li] = np.tile(np.asarray(b, np.float32)[None, :], (128, 1))

    per_core = []
    for c in range(cfg.ncores):
        sv = pp["sval"][c].astype(np.int64)                  # [cpb, 2*kb]
        bidx, eidx = np.nonzero(sv >= 0)
        t = eidx // 128
        e = eidx % 128
        S = np.zeros((cfg.cpb, 128, cfg.tpb * 128), dtype=np.float32)
        S[bidx, e, t * 128 + sv[bidx, eidx]] = 1.0
        ST = np.zeros((cfg.cpb, 128, cfg.tpb * 128), dtype=np.float32)
        ST[bidx, sv[bidx, eidx], eidx] = 1.0
        per_core.append(dict(
            xT_stack=xT_stack.astype(bfl),
            xT_local=np.ascontiguousarray(xT_stack[c * 128:(c + 1) * 128]).astype(bfl),
            w_ext=w_ext.astype(bfl), bias=bias,
            ident=np.eye(128, dtype=np.float32),
            idx1r=wrap_rep(pp["idx1"][c]),
            s_tiles=S.astype(bfl),
            st_tiles=ST.astype(bfl),
        ))
    return per_core


# -------------------------------------------------------------- device side

def build_nc(cfg):
    nc = bacc.Bacc("TRN2", num_devices=cfg.ncores)
    NPC, CPB, TPB, TB, KB = cfg.npc, cfg.cpb, cfg.tpb, cfg.t_bkt, cfg.kb
    NL = cfg.nlayers
    NSH = cfg.ncores          # shards
    HSH = NSH // 2            # shards per table half
    BSPL = (CPB + 1) // 2     # blocks in first AllGather column half
    C0 = BSPL * 128           # columns in first AG half

    xT_stack = nc.dram_tensor("xT_stack", [NSH * 128, NPC], f32r, kind="ExternalInput")
    xT_local = nc.dram_tensor("xT_local", [128, NPC], f32r, kind="ExternalInput")
    w_ext_in = nc.dram_tensor("w_ext", [NL, 128, ROW], f32r, kind="ExternalInput")
    bias_in = nc.dram_tensor("bias", [NL, 128, 128], f32, kind="ExternalInput")
    ident_in = nc.dram_tensor("ident", [128, 128], f32, kind="ExternalInput")
    idx1_in = nc.dram_tensor("idx1r", [128, CPB * 2 * KB // 16], i16, kind="ExternalInput")
    s_in = nc.dram_tensor("s_tiles", [CPB, 128, TPB * 128], f32r, kind="ExternalInput")
    st_in = nc.dram_tensor("st_tiles", [CPB, 128, TPB * 128], f32r, kind="ExternalInput")
    out = nc.dram_tensor("out", [NPC, 128], f32, kind="ExternalOutput")

    with tile.TileContext(nc) as tc:
        with (
            tc.tile_pool(name="const", bufs=1) as constp,
            tc.tile_pool(name="dram", bufs=2, space="DRAM") as dramp,
            tc.tile_pool(name="g1", bufs=4) as g1p,
            tc.tile_pool(name="sp", bufs=3) as sp,
            tc.tile_pool(name="stp", bufs=3) as stp,
            tc.tile_pool(name="att", bufs=8) as attp,
            tc.tile_pool(name="msgp", bufs=3) as msgp,
            tc.tile_pool(name="edslp", bufs=4) as edslp,
            tc.tile_pool(name="ep", bufs=3) as epp,
            tc.tile_pool(name="houtp", bufs=2) as houtp,
            tc.tile_pool(name="slabL", bufs=1) as slabLp,
        ):
            idx1_sb = constp.tile([128, CPB * 2 * KB // 16], i16)
            nc.sync.dma_start(idx1_sb[:], idx1_in.ap())
            w_sb = constp.tile([128, NL * ROW], f32r)
            bias_sb = constp.tile([128, NL * 128], f32)
            for li in range(NL):
                nc.sync.dma_start(w_sb[:, li * ROW:(li + 1) * ROW], w_ext_in.ap()[li])
                nc.sync.dma_start(bias_sb[:, li * 128:(li + 1) * 128], bias_in.ap()[li])
            ident_sb = constp.tile([128, 128], f32)
            nc.sync.dma_start(ident_sb[:], ident_in.ap())
            esed_sb = constp.tile([128, CPB * 4], f32r)

            greg = nc.gpsimd.to_reg(KB)

            # zero the gather pool slots once: tail slots beyond the bucket
            # fill keep stale SBUF contents; uninitialized bits could be NaN
            # bf16 patterns that poison 0*NaN in PSUM accumulation.
            for _ in range(4):
                gz = g1p.tile([128, TB, ROW], f32r, name="g1")
                nc.vector.memset(gz[:], 0)

            hTl0 = slabLp.tile([128, NPC], f32r)      # layer-0 local slab
            nc.sync.dma_start(hTl0[:], xT_local.ap())

            hag = None                # pair of DRAM AG outputs, or None
            prev_houtT = None         # previous layer's hT shard in SBUF

            for li in range(NL):
                w_l = w_sb[:, li * ROW:(li + 1) * ROW]
                bias_l = bias_sb[:, li * 128:(li + 1) * 128]
                last = li == NL - 1
                hT_local = hTl0 if li == 0 else prev_houtT

                table = [dramp.tile([HSH * NPC, ROW], f32r, tag=f"tab{h}",
                                    name=f"table_l{li}_h{h}")
                         for h in range(2)]

                # ---- phase A': local es/ed kept in SBUF
                with tc.tile_pool(name="psumE", bufs=4, space="PSUM") as psumEp:
                    for j in range(CPB):
                        psE = psumEp.tile([128, 4], f32)
                        nc.tensor.matmul(
                            psE[:],
                            hT_local[:, j * 128:(j + 1) * 128],
                            w_l[:, 256:260],
                            start=True, stop=True)
                        nc.scalar.activation(
                            esed_sb[:, j * 4:(j + 1) * 4], psE[:], ACTF.Copy)

                # ---- phase A: full table (replicated), batched narrow writes
                with (
                    tc.tile_pool(name="slabA", bufs=2) as slabAp,
                    tc.tile_pool(name="rowA", bufs=4) as rowAp,
                    tc.tile_pool(name="psumA", bufs=6, space="PSUM") as psumAp,
                ):
                    for s in range(NSH):
                        hTs = slabAp.tile([128, NPC], f32r)
                        if li == 0:
                            nc.sync.dma_start(
                                hTs[:], xT_stack.ap()[s * 128:(s + 1) * 128])
                        else:
                            nc.sync.dma_start(
                                hTs[:, 0:C0], hag[0][s * 128:(s + 1) * 128])
                            nc.sync.dma_start(
                                hTs[:, C0:NPC], hag[1][s * 128:(s + 1) * 128])
                        tab = table[s // HSH]
                        base = (s % HSH) * NPC
                        for j0 in range(0, CPB, 4):
                            g = min(4, CPB - j0)
                            rowA = rowAp.tile([128, 4, TCOL], f32r, name="rowA")
                            for jj in range(g):
                                psA = psumAp.tile([128, TCOL], f32)
                                nc.tensor.matmul(
                                    psA[:],
                                    hTs[:, (j0 + jj) * 128:(j0 + jj + 1) * 128],
                                    w_l[:, 0:TCOL],
                                    start=True, stop=True)
                                if jj % 2 == 0:
                                    nc.scalar.activation(
                                        rowA[:, jj, :], psA[:], ACTF.Copy)
                                else:
                                    nc.vector.tensor_copy(rowA[:, jj, :], psA[:])
                            dst_ap = tab[base + j0 * 128:
                                         base + (j0 + g) * 128, 0:TCOL]
                            dst_ap = dst_ap.rearrange("(a p) c -> p a c", p=128)
                            nc.sync.dma_start(dst_ap, rowA[:, 0:g, :])

                # ---- phase B
                with (
                    tc.tile_pool(name="psumB", bufs=2, space="PSUM") as psumBp,
                    tc.tile_pool(name="psumT", bufs=2, space="PSUM") as psumTp,
                    tc.tile_pool(name="psumD", bufs=2, space="PSUM") as psumDp,
                    tc.tile_pool(name="psumL", bufs=2, space="PSUM") as psumLp,
                ):
                    houtT = None
                    if not last:
                        houtT = houtp.tile([128, NPC], f32r, name="houtT")
                    for b in range(CPB):
                        st_sb = stp.tile([128, TPB * 128], f32r, name="st_sb")
                        nc.sync.dma_start(st_sb[:], st_in.ap()[b])
                        # ed per edge slot: transposed-one-hot matmuls
                        psD = psumDp.tile([128, TPB, 2], f32)
                        for t in range(TPB):
                            nc.tensor.matmul(
                                psD[:, t, :],
                                st_sb[:, t * 128:(t + 1) * 128],
                                esed_sb[:, b * 4 + 2:b * 4 + 4],
                                start=True, stop=True)
                        edsl = edslp.tile([128, TPB, 2], f32, name="edsl")
                        nc.scalar.activation(edsl[:], psD[:], ACTF.Copy)
                        # self-loop features of this block's dst nodes
                        psL = psumLp.tile([128, 256], f32)
                        nc.tensor.matmul(
                            psL[:],
                            hT_local[:, b * 128:(b + 1) * 128],
                            w_l[:, 0:256],
                            start=True, stop=True)

                        psum = psumBp.tile([128, 258], f32)
                        for h in range(2):
                            s_sb = sp.tile([128, TB * 128], f32r, name="s_sb")
                            nc.sync.dma_start(
                                s_sb[:],
                                s_in.ap()[b, :, h * TB * 128:(h + 1) * TB * 128])
                            g1 = g1p.tile([128, TB, ROW], f32r, name="g1")
                            nc.gpsimd.dma_gather(
                                out_ap=g1[:],
                                in_ap=table[h],
                                idxs_ap=idx1_sb[:, (b * 2 + h) * KB // 16:
                                                (b * 2 + h + 1) * KB // 16],
                                num_idxs=KB, num_idxs_reg=greg,
                                elem_size=ROW, single_packet=False)
                            # attention scalars (batched across TB tiles)
                            tat = attp.tile([128, TB, 2], f32, tag="tat")
                            nc.vector.tensor_tensor(
                                out=tat[:], in0=g1[:, :, 256:258],
                                in1=edsl[:, h * TB:(h + 1) * TB, :], op=ALU.add)
                            lk = attp.tile([128, TB, 2], f32, tag="lk")
                            nc.vector.tensor_scalar(
                                out=lk[:], in0=tat[:], scalar1=0.2,
                                scalar2=None, op0=ALU.mult)
                            nc.vector.tensor_tensor(
                                out=lk[:], in0=lk[:], in1=tat[:], op=ALU.max)
                            exe = attp.tile([128, TB, 2], f32, tag="exe")
                            nc.scalar.activation(exe[:], lk[:], ACTF.Exp)
                            exb = attp.tile([128, TB, 2], f32r, tag="exb")
                            nc.vector.tensor_copy(exb[:], exe[:])
                            msg = msgp.tile([128, TB, 258], f32r, name="msg")
                            for hh in range(2):
                                nc.vector.tensor_tensor(
                                    out=msg[:, :, hh * 128:(hh + 1) * 128],
                                    in0=g1[:, :, hh * 128:(hh + 1) * 128],
                                    in1=exb[:, :, hh:hh + 1].broadcast_to(
                                        (128, TB, 128)),
                                    op=ALU.mult)
                            nc.scalar.activation(
                                msg[:, :, 256:258], exe[:], ACTF.Copy)
                            for t in range(TB):
                                nc.tensor.matmul(
                                    psum[:],
                                    s_sb[:, t * 128:(t + 1) * 128],
                                    msg[:, t, :],
                                    start=(h == 0 and t == 0),
                                    stop=(h == 1 and t == TB - 1))
                        # ---- epilogue with self-loop merge
                        tsl = attp.tile([128, 2], f32, tag="tsl")
                        nc.vector.tensor_tensor(
                            out=tsl[:], in0=esed_sb[:, b * 4:b * 4 + 2],
                            in1=esed_sb[:, b * 4 + 2:b * 4 + 4], op=ALU.add)
                        lk2 = attp.tile([128, 2], f32, tag="lk2")
                        nc.vector.tensor_scalar(
                            out=lk2[:], in0=tsl[:], scalar1=0.2,
                            scalar2=None, op0=ALU.mult)
                        nc.vector.tensor_tensor(
                            out=lk2[:], in0=lk2[:], in1=tsl[:], op=ALU.max)
                        exs = attp.tile([128, 2], f32, tag="exs")
                        nc.scalar.activation(exs[:], lk2[:], ACTF.Exp)
                        den = attp.tile([128, 2], f32, tag="den")
                        nc.vector.tensor_tensor(
                            out=den[:], in0=psum[:, 256:258], in1=exs[:],
                            op=ALU.add)
                        rec = attp.tile([128, 2], f32, tag="rec")
                        nc.vector.tensor_scalar(
                            out=rec[:], in0=den[:], scalar1=2.0, scalar2=2e-30,
                            op0=ALU.mult, op1=ALU.add)
                        nc.vector.reciprocal(rec[:], rec[:])     # 0.5/den
                        z = epp.tile([128, 128], f32, tag="z")
                        for hh in range(2):
                            u = epp.tile([128, 128], f32, tag=f"u{hh}")
                            nc.vector.tensor_scalar(
                                out=u[:], in0=psL[:, hh * 128:(hh + 1) * 128],
                                scalar1=exs[:, hh:hh + 1], scalar2=None,
                                op0=ALU.mult)
                            nc.vector.tensor_tensor(
                                out=u[:], in0=u[:],
                                in1=psum[:, hh * 128:(hh + 1) * 128],
                                op=ALU.add)
                            # u *= rec_hh (0.5/den)
                            nc.scalar.activation(
                                u[:], u[:], ACTF.Copy, scale=rec[:, hh:hh + 1])
                            if hh == 0:
                                nc.vector.tensor_tensor(
                                    out=z[:], in0=u[:], in1=bias_l, op=ALU.add)
                            else:
                                nc.vector.tensor_tensor(
                                    out=z[:], in0=z[:], in1=u[:], op=ALU.add)
                        if not last:
                            # ELU = relu(z) + exp(min(z,0)) - 1
                            rz = epp.tile([128, 128], f32, tag="rz")
                            nc.scalar.activation(rz[:], z[:], ACTF.Relu)
                            rn = epp.tile([128, 128], f32, tag="rn")
                            nc.scalar.activation(rn[:], z[:], ACTF.Relu,
                                                 scale=-1.0)
                            emn = epp.tile([128, 128], f32, tag="emn")
                            nc.scalar.activation(emn[:], rn[:], ACTF.Exp,
                                                 scale=-1.0)
                            hb = epp.tile([128, 128], f32, tag="hb")
                            nc.vector.tensor_tensor(
                                out=hb[:], in0=rz[:], in1=emn[:], op=ALU.add)
                            nc.vector.tensor_scalar(
                                out=hb[:], in0=hb[:], scalar1=-1.0,
                                scalar2=None, op0=ALU.add)
                            psT = psumTp.tile([128, 128], f32)
                            nc.tensor.transpose(psT[:], hb[:], ident_sb[:])
                            nc.scalar.activation(
                                houtT[:, b * 128:(b + 1) * 128], psT[:],
                                ACTF.Copy)
                            if b == BSPL - 1:
                                hloc0 = dramp.tile([128, C0], f32r, tag="hl0")
                                nc.sync.dma_start(hloc0[:], houtT[:, 0:C0])
                                hag0 = dramp.tile([NSH * 128, C0], f32r,
                                                  tag="hag0", addr_space="Shared")
                                nc.gpsimd.collective_compute(
                                    "AllGather", ALU.bypass,
                                    replica_groups=[list(range(cfg.ncores))],
                                    ins=[hloc0.opt()], outs=[hag0.opt()])
                        else:
                            nc.sync.dma_start(
                                out[b * 128:(b + 1) * 128, :], z[:])
                    if not last:
                        hloc1 = dramp.tile([128, NPC - C0], f32r, tag="hl1")
                        nc.sync.dma_start(hloc1[:], houtT[:, C0:NPC])
                        hag1 = dramp.tile([NSH * 128, NPC - C0], f32r,
                                          tag="hag1", addr_space="Shared")
                        nc.gpsimd.collective_compute(
                            "AllGather", ALU.bypass,
                            replica_groups=[list(range(cfg.ncores))],
                            ins=[hloc1.opt()], outs=[hag1.opt()])
                        hag = (hag0, hag1)
                        prev_houtT = houtT
    nc.compile()
    return nc


# ------------------------------------------------------------------ driver

def run(cfg, x, edge_index, params, trace=False):
    from concourse.bass_utils import run_bass_kernel_spmd
    probe = preprocess(dataclasses.replace(cfg, t_bkt=64), edge_index)
    cfg = dataclasses.replace(cfg, t_bkt=max(probe["t_need"], 1))
    pp = preprocess(cfg, edge_index)
    per_core = host_arrays(cfg, pp, x, params)
    nc = build_nc(cfg)
    in_maps = [
        dict(xT_stack=pc["xT_stack"], xT_local=pc["xT_local"],
             w_ext=pc["w_ext"], bias=pc["bias"], ident=pc["ident"],
             idx1r=pc["idx1r"], s_tiles=pc["s_tiles"], st_tiles=pc["st_tiles"])
        for pc in per_core
    ]
    res = run_bass_kernel_spmd(
        nc, in_maps, core_ids=list(range(cfg.ncores)), trace=trace)
    full = np.concatenate([res.results[c]["out"] for c in range(cfg.ncores)])
    return full[pp["perm"]], res


# ------------------------------------------------------------- entry point

_CFG = Cfg()


def kernel(x, edge_index, W0, a_src0, a_dst0, b0, W1, a_src1, a_dst1, b1,
           W2, a_src2, a_dst2, b2):
    """Full-input GAT kernel: shards across 8 NeuronCores internally."""
    params = [(W0, a_src0, a_dst0, b0), (W1, a_src1, a_dst1, b1),
              (W2, a_src2, a_dst2, b2)]
    out, _ = run(_CFG, x, edge_index, params, trace=False)
    return np.asarray(out, dtype=np.float32)


# revision 22
# speedup vs baseline: 1.8406x; 1.0565x over previous
"""3-layer GAT (2 heads x 128) on 8 TRN2 NeuronCores — Bass/Tile kernel.

Sharding: nodes partitioned across cores by destination (graph parallel);
weights replicated; per-layer split AllGather of transposed features.

Device algorithm per layer:
  phase A' (local): esed_sb[:, j, :] = hT_local_chunk.T @ W_ext[:,256:260]
    kept in SBUF (per-node es/ed of the local shard).
  phase A (replicated): table rows [xh(256), es(2)] (258 of ROW=384, bf16)
    written to local DRAM in 4-chunk batched DMAs.
  phase B (sharded, per 128-dst block):
    ed per edge slot via transposed-one-hot PE matmuls (ST tiles);
    dma_gather table rows by src (2 src-half buckets, int16 idx);
    ex = exp(max(t,.2t)); msg = ex*xh; PE matmul with one-hot S tiles
    accumulates [128 dst, 258] (agg heads + denominators). Self-loop
    edges never gathered: xh_local from a direct PE matmul, merged in
    the epilogue. out = (agg0/den0 + agg1/den1)/2 + b (+ELU);
    PE-transpose -> hT shard kept in SBUF (houtT, double-buffered).
  AllGather of hT split in two column halves; the first overlaps the
  tail of phase B. Softmax max-subtraction is skipped (logits are O(1);
  exp is safe in fp32).
"""
import dataclasses
import numpy as np

import concourse.bass as bass
import concourse.bacc as bacc
import concourse.mybir as mybir
import concourse.tile as tile

f32 = mybir.dt.float32
f32r = mybir.dt.bfloat16  # bf16 matmul operands (fp32r broken on HW)
i16 = mybir.dt.int16
ALU = mybir.AluOpType
ACTF = mybir.ActivationFunctionType


@dataclasses.dataclass(frozen=True)
class Cfg:
    n: int = 50000
    ncores: int = 8
    t_bkt: int = 9
    nlayers: int = 3
    hid: int = 128          # per-head dim == in feat dim == 128 (fixed)

    @property
    def nb(self):  return self.n // self.ncores
    @property
    def cpb(self):  return (self.nb + 127) // 128
    @property
    def npc(self):  return self.cpb * 128
    @property
    def npad(self): return self.ncores * self.npc
    @property
    def half(self): return self.npad // 2
    @property
    def tpb(self):  return 2 * self.t_bkt
    @property
    def kb(self):   return self.t_bkt * 128


ROW = 384
TCOL = 258                  # meaningful table cols: xh(256) + es(2)


# ---------------------------------------------------------------- host side

def pack_nodes(cfg, deg):
    """perm [N] -> padded slot id. Cores by contiguous range; within a core,
    degree-sorted snake deal into cpb blocks (balances block edge counts)."""
    perm = np.full(cfg.n, -1, dtype=np.int64)
    for c in range(cfg.ncores):
        nodes = np.arange(c * cfg.nb, (c + 1) * cfg.nb)
        order = nodes[np.argsort(-deg[nodes], kind="stable")]
        blk = np.empty(cfg.nb, dtype=np.int64)
        slot = np.empty(cfg.nb, dtype=np.int64)
        fr = cfg.nb // cfg.cpb
        rem = cfg.nb - fr * cfg.cpb
        for r in range(fr):
            cols = np.arange(cfg.cpb)
            if r % 2:
                cols = cols[::-1]
            blk[r * cfg.cpb:(r + 1) * cfg.cpb] = cols
            slot[r * cfg.cpb:(r + 1) * cfg.cpb] = r
        if rem:
            cols = np.arange(rem) if fr % 2 == 0 else (cfg.cpb - 1 - np.arange(rem))
            blk[fr * cfg.cpb:] = cols
            slot[fr * cfg.cpb:] = fr
        perm[order] = c * cfg.npc + blk * 128 + slot
    return perm


def preprocess(cfg, edge_index):
    # self-loops (appended by the reference for every node) are handled by
    # a local compute path on device, NOT via gather buckets.
    src = np.asarray(edge_index[0], dtype=np.int64)
    dst = np.asarray(edge_index[1], dtype=np.int64)

    deg = np.bincount(dst, minlength=cfg.n)
    perm = pack_nodes(cfg, deg)

    psrc = perm[src]
    pdst = perm[dst]
    core = pdst // cfg.npc
    blk = (pdst % cfg.npc) // 128
    half = (psrc >= cfg.half).astype(np.int64)

    order = np.lexsort((psrc, half, blk, core))
    psrc, pdst, half = psrc[order], pdst[order], half[order]
    group = (core * cfg.cpb + blk)[order] * 2 + half

    ngroups = cfg.ncores * cfg.cpb * 2
    cnt = np.bincount(group, minlength=ngroups)
    t_need = int((cnt.max() + 127) // 128)
    assert cfg.t_bkt >= t_need, f"t_bkt={cfg.t_bkt} < needed {t_need}"
    starts = np.zeros(ngroups + 1, dtype=np.int64)
    np.cumsum(cnt, out=starts[1:])
    within = np.arange(len(group)) - starts[group]
    gpos = group * cfg.kb + within

    idx1 = np.zeros(ngroups * cfg.kb, dtype=np.int16)
    idx1[gpos] = (psrc - half * cfg.half).astype(np.int16)
    idx1 = idx1.reshape(cfg.ncores, cfg.cpb, 2 * cfg.kb)

    sval = np.full(ngroups * cfg.kb, -1, dtype=np.int16)
    sval[gpos] = (pdst % 128).astype(np.int16)
    sval = sval.reshape(cfg.ncores, cfg.cpb, 2 * cfg.kb)
    return dict(perm=perm, t_need=t_need, idx1=idx1, sval=sval)


def wrap_rep(idx):
    """[..., K] int16 -> dma_gather wrapped layout [128, prod*K/16]."""
    K = idx.shape[-1]
    lead = int(np.prod(idx.shape[:-1]))
    w = idx.reshape(lead, K // 16, 16)
    w = np.transpose(w, (2, 0, 1)).reshape(16, lead * (K // 16))
    return np.tile(w, (8, 1)).copy()


def host_arrays(cfg, pp, x, params):
    import ml_dtypes
    bfl = ml_dtypes.bfloat16
    perm = pp["perm"]

    xpad = np.zeros((cfg.npad, 128), dtype=np.float32)
    xpad[perm] = np.asarray(x, np.float32)
    xT_stack = np.ascontiguousarray(
        xpad.reshape(cfg.ncores, cfg.npc, 128).transpose(0, 2, 1)
        .reshape(cfg.ncores * 128, cfg.npc))

    w_ext = np.zeros((cfg.nlayers, 128, ROW), dtype=np.float32)
    bias = np.zeros((cfg.nlayers, 128, 128), dtype=np.float32)
    for li, (W, a_s, a_d, b) in enumerate(params):
        W = np.asarray(W, np.float32)
        w_ext[li, :, :256] = W
        w_ext[li, :, 256] = W[:, :128] @ np.asarray(a_s, np.float32)[0]
        w_ext[li, :, 257] = W[:, 128:] @ np.asarray(a_s, np.float32)[1]
        w_ext[li, :, 258] = W[:, :128] @ np.asarray(a_d, np.float32)[0]
        w_ext[li, :, 259] = W[:, 128:] @ np.asarray(a_d, np.float32)[1]
        bias[li] = np.tile(np.asarray(b, np.float32)[None, :], (128, 1))

    per_core = []
    for c in range(cfg.ncores):
        sv = pp["sval"][c].astype(np.int64)                  # [cpb, 2*kb]
        bidx, eidx = np.nonzero(sv >= 0)
        t = eidx // 128
        e = eidx % 128
        S = np.zeros((cfg.cpb, 128, cfg.tpb * 128), dtype=np.float32)
        S[bidx, e, t * 128 + sv[bidx, eidx]] = 1.0
        ST = np.zeros((cfg.cpb, 128, cfg.tpb * 128), dtype=np.float32)
        ST[bidx, sv[bidx, eidx], eidx] = 1.0
        per_core.append(dict(
            xT_stack=xT_stack.astype(bfl),
            xT_local=np.ascontiguousarray(xT_stack[c * 128:(c + 1) * 128]).astype(bfl),
            w_ext=w_ext.astype(bfl), bias=bias,
            ident=np.eye(128, dtype=np.float32),
            idx1r=wrap_rep(pp["idx1"][c]),
            s_tiles=S.astype(bfl),
            st_tiles=ST.astype(bfl),
        ))
    return per_core


# -------------------------------------------------------------- device side

def build_nc(cfg):
    nc = bacc.Bacc("TRN2", num_devices=cfg.ncores)
    NPC, CPB, TPB, TB, KB = cfg.npc, cfg.cpb, cfg.tpb, cfg.t_bkt, cfg.kb
    NL = cfg.nlayers
    NSH = cfg.ncores          # shards
    HSH = NSH // 2            # shards per table half
    # AllGather pieces: (fire_after_block_idx, col_lo, col_hi)
    AGS = [(19, 0, 20 * 128), (35, 20 * 128, 36 * 128),
           (CPB - 1, 36 * 128, NPC)]

    xT_stack = nc.dram_tensor("xT_stack", [NSH * 128, NPC], f32r, kind="ExternalInput")
    xT_local = nc.dram_tensor("xT_local", [128, NPC], f32r, kind="ExternalInput")
    w_ext_in = nc.dram_tensor("w_ext", [NL, 128, ROW], f32r, kind="ExternalInput")
    bias_in = nc.dram_tensor("bias", [NL, 128, 128], f32, kind="ExternalInput")
    ident_in = nc.dram_tensor("ident", [128, 128], f32, kind="ExternalInput")
    idx1_in = nc.dram_tensor("idx1r", [128, CPB * 2 * KB // 16], i16, kind="ExternalInput")
    s_in = nc.dram_tensor("s_tiles", [CPB, 128, TPB * 128], f32r, kind="ExternalInput")
    st_in = nc.dram_tensor("st_tiles", [CPB, 128, TPB * 128], f32r, kind="ExternalInput")
    out = nc.dram_tensor("out", [NPC, 128], f32, kind="ExternalOutput")

    with tile.TileContext(nc) as tc:
        with (
            tc.tile_pool(name="const", bufs=1) as constp,
            tc.tile_pool(name="dram", bufs=2, space="DRAM") as dramp,
            tc.tile_pool(name="g1", bufs=4) as g1p,
            tc.tile_pool(name="sp", bufs=3) as sp,
            tc.tile_pool(name="stp", bufs=3) as stp,
            tc.tile_pool(name="att", bufs=8) as attp,
            tc.tile_pool(name="edslp", bufs=4) as edslp,
            tc.tile_pool(name="ep", bufs=3) as epp,
            tc.tile_pool(name="diagp", bufs=3) as diagp,
            tc.tile_pool(name="xhp", bufs=3) as xhp,
            tc.tile_pool(name="houtp", bufs=2) as houtp,
            tc.tile_pool(name="slabL", bufs=1) as slabLp,
        ):
            idx1_sb = constp.tile([128, CPB * 2 * KB // 16], i16)
            nc.sync.dma_start(idx1_sb[:], idx1_in.ap())
            w_sb = constp.tile([128, NL * ROW], f32r)
            bias_sb = constp.tile([128, NL * 128], f32)
            for li in range(NL):
                nc.sync.dma_start(w_sb[:, li * ROW:(li + 1) * ROW], w_ext_in.ap()[li])
                nc.sync.dma_start(bias_sb[:, li * 128:(li + 1) * 128], bias_in.ap()[li])
            ident_sb = constp.tile([128, 128], f32)
            nc.sync.dma_start(ident_sb[:], ident_in.ap())
            esed_es = constp.tile([128, CPB * 2], f32r)
            esed_ed = constp.tile([128, CPB * 2], f32r)
            exs_all = constp.tile([128, CPB * 2], f32)

            greg = nc.gpsimd.to_reg(KB)

            # zero the gather pool slots once: tail slots beyond the bucket
            # fill keep stale SBUF contents; uninitialized bits could be NaN
            # bf16 patterns that poison 0*NaN in PSUM accumulation.
            for _ in range(6):
                gz = g1p.tile([128, TB, ROW], f32r, name="g1")
                nc.vector.memset(gz[:], 0)


            hTl0 = slabLp.tile([128, NPC], f32r)      # layer-0 local slab
            nc.sync.dma_start(hTl0[:], xT_local.ap())

            hag = None                # pair of DRAM AG outputs, or None
            prev_houtT = None         # previous layer's hT shard in SBUF

            for li in range(NL):
                w_l = w_sb[:, li * ROW:(li + 1) * ROW]
                bias_l = bias_sb[:, li * 128:(li + 1) * 128]
                last = li == NL - 1
                hT_local = hTl0 if li == 0 else prev_houtT

                table = [dramp.tile([HSH * NPC, ROW], f32r, tag=f"tab{h}",
                                    name=f"table_l{li}_h{h}")
                         for h in range(2)]

                # ---- phase A': local es/ed kept in SBUF
                with tc.tile_pool(name="psumE", bufs=4, space="PSUM") as psumEp:
                    for j in range(CPB):
                        psE = psumEp.tile([128, 4], f32)
                        nc.tensor.matmul(
                            psE[:],
                            hT_local[:, j * 128:(j + 1) * 128],
                            w_l[:, 256:260],
                            start=True, stop=True)
                        nc.scalar.activation(
                            esed_es[:, j * 2:(j + 1) * 2], psE[:, 0:2],
                            ACTF.Copy)
                        nc.scalar.activation(
                            esed_ed[:, j * 2:(j + 1) * 2], psE[:, 2:4],
                            ACTF.Copy)
                    # self-loop attention exp for all blocks at once
                    tslA = attp.tile([128, CPB * 2], f32, tag="tslA")
                    nc.vector.tensor_tensor(
                        out=tslA[:], in0=esed_es[:],
                        in1=esed_ed[:], op=ALU.add)
                    lkA = attp.tile([128, CPB * 2], f32, tag="lkA")
                    nc.vector.tensor_scalar(
                        out=lkA[:], in0=tslA[:], scalar1=0.2,
                        scalar2=None, op0=ALU.mult)
                    nc.vector.tensor_tensor(
                        out=lkA[:], in0=lkA[:], in1=tslA[:], op=ALU.max)
                    nc.scalar.activation(exs_all[:], lkA[:], ACTF.Exp)

                # ---- phase A: full table (replicated), batched narrow writes
                with (
                    tc.tile_pool(name="slabA", bufs=2) as slabAp,
                    tc.tile_pool(name="rowA", bufs=4) as rowAp,
                    tc.tile_pool(name="psumA", bufs=6, space="PSUM") as psumAp,
                ):
                    for s in range(NSH):
                        hTs = slabAp.tile([128, NPC], f32r)
                        if li == 0:
                            nc.sync.dma_start(
                                hTs[:], xT_stack.ap()[s * 128:(s + 1) * 128])
                        else:
                            for pi, (_, lo, hi) in enumerate(AGS):
                                nc.sync.dma_start(
                                    hTs[:, lo:hi],
                                    hag[pi][s * 128:(s + 1) * 128])
                        tab = table[s // HSH]
                        base = (s % HSH) * NPC
                        for j0 in range(0, CPB, 4):
                            g = min(4, CPB - j0)
                            rowA = rowAp.tile([128, 4, TCOL], f32r, name="rowA")
                            for jj in range(g):
                                psA = psumAp.tile([128, TCOL], f32)
                                nc.tensor.matmul(
                                    psA[:],
                                    hTs[:, (j0 + jj) * 128:(j0 + jj + 1) * 128],
                                    w_l[:, 0:TCOL],
                                    start=True, stop=True)
                                if jj % 2 == 0:
                                    nc.scalar.activation(
                                        rowA[:, jj, :], psA[:], ACTF.Copy)
                                else:
                                    nc.vector.tensor_copy(rowA[:, jj, :], psA[:])
                            dst_ap = tab[base + j0 * 128:
                                         base + (j0 + g) * 128, 0:TCOL]
                            dst_ap = dst_ap.rearrange("(a p) c -> p a c", p=128)
                            nc.sync.dma_start(dst_ap, rowA[:, 0:g, :])

                # ---- phase B
                with (
                    tc.tile_pool(name="psumB", bufs=3, space="PSUM") as psumBp,
                    tc.tile_pool(name="psumT", bufs=1, space="PSUM") as psumTp,
                    tc.tile_pool(name="psumD", bufs=2, space="PSUM") as psumDp,
                    tc.tile_pool(name="psumL", bufs=2, space="PSUM") as psumLp,
                ):
                    houtT = None
                    ag_pieces = []
                    if not last:
                        houtT = houtp.tile([128, NPC], f32r, name="houtT")
                    for b in range(CPB):
                        st_sb = stp.tile([128, TPB * 128], f32r, name="st_sb")
                        nc.sync.dma_start(st_sb[:], st_in.ap()[b])
                        # ed per edge slot: transposed-one-hot matmuls
                        psD = psumDp.tile([128, TPB, 2], f32)
                        for t in range(TPB):
                            nc.tensor.matmul(
                                psD[:, t, :],
                                st_sb[:, t * 128:(t + 1) * 128],
                                esed_ed[:, b * 2:(b + 1) * 2],
                                start=True, stop=True)
                        edsl = edslp.tile([128, TPB, 2], f32, name="edsl")
                        nc.scalar.activation(edsl[:], psD[:], ACTF.Copy)
                        # self-loop features of this block's dst nodes
                        psL = psumLp.tile([128, 256], f32)
                        nc.tensor.matmul(
                            psL[:],
                            hT_local[:, b * 128:(b + 1) * 128],
                            w_l[:, 0:256],
                            start=True, stop=True)

                        # self-loop rhs: xh_local in SBUF + ones cols for den
                        xh_sb = xhp.tile([128, 258], f32r, name="xh_sb")
                        nc.scalar.activation(xh_sb[:, 0:256], psL[:], ACTF.Copy)
                        nc.vector.memset(xh_sb[:, 256:258], 1.0)
                        diags = []
                        for hh in range(2):
                            dg = diagp.tile([128, 128], f32r, tag=f"diag{hh}")
                            nc.vector.tensor_scalar(
                                out=dg[:], in0=ident_sb[:],
                                scalar1=exs_all[:, b * 2 + hh:b * 2 + hh + 1],
                                scalar2=None, op0=ALU.mult)
                            diags.append(dg)

                        psum = psumBp.tile([128, 258], f32)
                        for h in range(2):
                            s_sb = sp.tile([128, TB * 128], f32r, name="s_sb")
                            nc.sync.dma_start(
                                s_sb[:],
                                s_in.ap()[b, :, h * TB * 128:(h + 1) * TB * 128])
                            g1 = g1p.tile([128, TB, ROW], f32r, name="g1")
                            nc.gpsimd.dma_gather(
                                out_ap=g1[:],
                                in_ap=table[h],
                                idxs_ap=idx1_sb[:, (b * 2 + h) * KB // 16:
                                                (b * 2 + h + 1) * KB // 16],
                                num_idxs=KB, num_idxs_reg=greg,
                                elem_size=ROW, single_packet=False)
                            # attention scalars (batched across TB tiles)
                            tat = attp.tile([128, TB, 2], f32, tag="tat")
                            nc.vector.tensor_tensor(
                                out=tat[:], in0=g1[:, :, 256:258],
                                in1=edsl[:, h * TB:(h + 1) * TB, :], op=ALU.add)
                            lk = attp.tile([128, TB, 2], f32, tag="lk")
                            nc.vector.tensor_scalar(
                                out=lk[:], in0=tat[:], scalar1=0.2,
                                scalar2=None, op0=ALU.mult)
                            nc.vector.tensor_tensor(
                                out=lk[:], in0=lk[:], in1=tat[:], op=ALU.max)
                            exe = attp.tile([128, TB, 2], f32, tag="exe")
                            nc.scalar.activation(exe[:], lk[:], ACTF.Exp)
                            exb = attp.tile([128, TB, 2], f32r, tag="exb")
                            nc.vector.tensor_copy(exb[:], exe[:])
                            # scale gathered rows in place: msg = ex * xh
                            for hh in range(2):
                                nc.vector.tensor_tensor(
                                    out=g1[:, :, hh * 128:(hh + 1) * 128],
                                    in0=g1[:, :, hh * 128:(hh + 1) * 128],
                                    in1=exb[:, :, hh:hh + 1].broadcast_to(
                                        (128, TB, 128)),
                                    op=ALU.mult)
                            nc.scalar.activation(
                                g1[:, :, 256:258], exe[:], ACTF.Copy)
                            for t in range(TB):
                                nc.tensor.matmul(
                                    psum[:],
                                    s_sb[:, t * 128:(t + 1) * 128],
                                    g1[:, t, 0:258],
                                    start=(h == 0 and t == 0),
                                    stop=False)
                        # self-loop fold: psum += diag(exs) @ [xh_local | 1]
                        for hh in range(2):
                            nc.tensor.matmul(
                                psum[:, hh * 128:(hh + 1) * 128],
                                diags[hh], xh_sb[:, hh * 128:(hh + 1) * 128],
                                start=False, stop=False)
                            nc.tensor.matmul(
                                psum[:, 256 + hh:257 + hh],
                                diags[hh], xh_sb[:, 256:257],
                                start=False, stop=(hh == 1))
                        # ---- epilogue
                        rec = attp.tile([128, 2], f32, tag="rec")
                        nc.vector.tensor_scalar(
                            out=rec[:], in0=psum[:, 256:258], scalar1=2.0,
                            scalar2=2e-30, op0=ALU.mult, op1=ALU.add)
                        nc.vector.reciprocal(rec[:], rec[:])     # 0.5/den
                        z = epp.tile([128, 128], f32, tag="z")
                        for hh in range(2):
                            u = epp.tile([128, 128], f32, tag=f"u{hh}")
                            nc.scalar.activation(
                                u[:], psum[:, hh * 128:(hh + 1) * 128],
                                ACTF.Copy, scale=rec[:, hh:hh + 1])
                            if hh == 0:
                                nc.vector.tensor_tensor(
                                    out=z[:], in0=u[:], in1=bias_l, op=ALU.add)
                            else:
                                nc.vector.tensor_tensor(
                                    out=z[:], in0=z[:], in1=u[:], op=ALU.add)
                        if not last:
                            # ELU = relu(z) + exp(min(z,0)) - 1
                            rz = epp.tile([128, 128], f32, tag="rz")
                            nc.scalar.activation(rz[:], z[:], ACTF.Relu)
                            rn = epp.tile([128, 128], f32, tag="rn")
                            nc.scalar.activation(rn[:], z[:], ACTF.Relu,
                                                 scale=-1.0)
                            emn = epp.tile([128, 128], f32, tag="emn")
                            nc.scalar.activation(emn[:], rn[:], ACTF.Exp,
                                                 scale=-1.0)
                            hb = epp.tile([128, 128], f32, tag="hb")
                            nc.vector.tensor_tensor(
                                out=hb[:], in0=rz[:], in1=emn[:], op=ALU.add)
                            nc.vector.tensor_scalar(
                                out=hb[:], in0=hb[:], scalar1=-1.0,
                                scalar2=None, op0=ALU.add)
                            psT = psumTp.tile([128, 128], f32)
                            nc.tensor.transpose(psT[:], hb[:], ident_sb[:])
                            nc.scalar.activation(
                                houtT[:, b * 128:(b + 1) * 128], psT[:],
                                ACTF.Copy)
                            for pi, (fb, lo, hi) in enumerate(AGS):
                                if b == fb:
                                    hl = dramp.tile([128, hi - lo], f32r,
                                                    tag=f"hl{pi}")
                                    nc.sync.dma_start(hl[:], houtT[:, lo:hi])
                                    hg = dramp.tile(
                                        [NSH * 128, hi - lo], f32r,
                                        tag=f"hag{pi}", addr_space="Shared")
                                    nc.gpsimd.collective_compute(
                                        "AllGather", ALU.bypass,
                                        replica_groups=[list(range(cfg.ncores))],
                                        ins=[hl.opt()], outs=[hg.opt()])
                                    ag_pieces.append(hg)
                        else:
                            nc.sync.dma_start(
                                out[b * 128:(b + 1) * 128, :], z[:])
                    if not last:
                        hag = tuple(ag_pieces)
                        prev_houtT = houtT
    nc.compile()
    return nc


# ------------------------------------------------------------------ driver

def run(cfg, x, edge_index, params, trace=False):
    from concourse.bass_utils import run_bass_kernel_spmd
    probe = preprocess(dataclasses.replace(cfg, t_bkt=64), edge_index)
    cfg = dataclasses.replace(cfg, t_bkt=max(probe["t_need"], 1))
    pp = preprocess(cfg, edge_index)
    per_core = host_arrays(cfg, pp, x, params)
    nc = build_nc(cfg)
    in_maps = [
        dict(xT_stack=pc["xT_stack"], xT_local=pc["xT_local"],
             w_ext=pc["w_ext"], bias=pc["bias"], ident=pc["ident"],
             idx1r=pc["idx1r"], s_tiles=pc["s_tiles"], st_tiles=pc["st_tiles"])
        for pc in per_core
    ]
    res = run_bass_kernel_spmd(
        nc, in_maps, core_ids=list(range(cfg.ncores)), trace=trace)
    full = np.concatenate([res.results[c]["out"] for c in range(cfg.ncores)])
    return full[pp["perm"]], res


# ------------------------------------------------------------- entry point

_CFG = Cfg()


def kernel(x, edge_index, W0, a_src0, a_dst0, b0, W1, a_src1, a_dst1, b1,
           W2, a_src2, a_dst2, b2):
    """Full-input GAT kernel: shards across 8 NeuronCores internally."""
    params = [(W0, a_src0, a_dst0, b0), (W1, a_src1, a_dst1, b1),
              (W2, a_src2, a_dst2, b2)]
    out, _ = run(_CFG, x, edge_index, params, trace=False)
    return np.asarray(out, dtype=np.float32)
